# revision 27
# baseline (speedup 1.0000x reference)
"""CrossViewTransformer Trainium2 kernel.

Math (per batch b):
    q = Wq @ bev + bq          [D=8,  N=9216]
    k = Wk @ rv  + bk          [8,  N]
    v = Wv @ rv  + bv          [64, N]
    E[j, i] = k[:, j] . q[:, i]            (energy, rows=key pixel j, cols=query pixel i)
    A = softmax over i of E[j, :]
    z[:, j] = sum_i A[j, i] * v[:, i]
    out = bev + z

Sharding: 8 cores = 2 batches x 4 j-slabs of 2304 columns. Each core computes
softmax over the full i axis for its j slab; no collectives.

Device pipeline (per core):
  - Energy E^T tiles [i-chunk=128, jblk] via fp8 DoubleRow matmul (0.5
    cyc/col), q/k packed [4, 2, n] slot-major; f32 PSUM logits.
  - P = exp(E) in bf16: ScalarE real exp, or DVE Schraudolph fast-exp (one
    tensor_scalar to int16 = bf16 bit pattern). Strict ACT/DVE alternation
    keeps each mod-3 PSUM-rotation chain engine-mixed. (An fp8-P variant
    with DoubleRow z was tried: any schedule where BOTH engines emit 1-byte
    exp outputs crashes the exec unit - NRT_EXEC_UNIT_UNRECOVERABLE - while
    either engine alone at full scale is fine; P therefore stays 2-byte.)
  - z accumulation bf16: zt[j, c|denom] += matmul(lhsT=P^T[:, jsub],
    rhs=v^T_ext chunk), softmax denominator riding as vt's ones column.
  - Projections: q/k blocks land in PSUM at partition bases 0/32/64/96
    (tile_position), channel-interleaved so partition p = 2r+s; one 512-free
    PSUM->SBUF fp8 copy covers FOUR blocks (the 4x-replicated 32-wide lhsT
    writes whole quadrants so no PSUM byte is read uninitialized), then one
    SBUF->SBUF HWDGE remap DMA per 2-block strip scatters into the packed
    [4, 2, N] energy layout. Cuts the ACT/DVE projection-copy load ~3x vs
    per-block [4, 1024] copies. The first q/k blocks use direct copies (the
    ~2.2us remap-DMA latency exceeds their deadline). vt chunks batch 8 per
    PSUM tile (4 per bank) -> one 520-free copy each.
  - Flattened software pipeline (energy i | exp i-1 | z i-3) across jblocks;
    producer copies drip in AFTER energy(i) with the copy engine matched to
    exp(i+1)'s parity so both PSUM-rotation waits land in that engine's
    in-order queue. Dummy matmuls at t~1us start the PE p-state ramp early.
  - Epilogue per j-128-subblock: DVE reciprocal, ACT scale-copy to bf16,
    gpsimd residual add, bf16 store per jblock (immediate for the last).
"""

import sys

if "/opt/trn_rl_repo" not in sys.path:
    sys.path.insert(0, "/opt/trn_rl_repo")

import os

import numpy as np
import ml_dtypes

# exp engine pattern: ACT runs the first DVE_START groups solo (its solo
# cadence leaves the 3-deep energy-buffer rotation slack that absorbs all
# producer-copy hops) while DVE streams every projection copy; from
# DVE_START on, strict ACT/DVE alternation (keeps each mod-3 PSUM reuse
# chain engine-mixed). Every EXTRA_A-th DVE slot flips to ACT to rebalance.
EXTRA_A = int(os.environ.get("EXTRA_A", "0"))
DVE_START = int(os.environ.get("DVE_START", "1"))

B, C, H, W = 2, 64, 96, 96
N = H * W            # 9216
D = C // 8           # 8
NT = N // 128        # 72 i-chunks
NP = NT // 2         # 36 i-chunk pairs
NCORES = 8
JS = N // 4          # 2304 columns per core
JBLOCKS = [(0, 512), (512, 512), (1024, 512), (1536, 512), (2048, 256)]
GW = 1024            # exp-group width (elements per ACT/DVE call)

# bf16 Schraudolph fast-exp: bits16 = rint(x * 128/ln2 + B); int16 bit
# pattern reinterpreted as bf16 gives exp(x) to ~3% (HW-proven baseline
# path; mixing 1-byte fp8/int8 exp outputs across both engines crashes the
# exec unit, so P stays 2-byte).
EXP_A = float(128.0 / np.log(2.0))
EXP_B = 16256.0 - 5.0

BF16 = ml_dtypes.bfloat16
F8NP = ml_dtypes.float8_e4m3fn

_PROGRAMS = {}


def _build_program(reps=1, dve_share=None):
    extra_a, dve_start = (EXTRA_A, DVE_START) if dve_share is None else dve_share

    def exp_on_dve(i):
        if i < dve_start or (i - dve_start) % 2 == 1:
            return False
        if extra_a and ((i - dve_start) // 2) % extra_a == extra_a - 1:
            return False
        return True
    import concourse.bacc as bacc
    import concourse.mybir as mybir
    from concourse import tile

    F32 = mybir.dt.float32
    BF = mybir.dt.bfloat16
    F8 = mybir.dt.float8e4
    I16 = mybir.dt.int16
    Exp = mybir.ActivationFunctionType.Exp
    Copy = mybir.ActivationFunctionType.Copy
    Mul = mybir.AluOpType.mult
    Add = mybir.AluOpType.add
    DR = mybir.MatmulPerfMode.DoubleRow

    f32epi = os.environ.get("F32_EPI", "") == "1"
    nc = bacc.Bacc("TRN2", target_bir_lowering=False, num_devices=NCORES)

    rv_d = nc.dram_tensor("rv_ext", [65, N], BF, kind="ExternalInput")
    bev_d = nc.dram_tensor("bev_ext", [65, N], BF, kind="ExternalInput")
    rvs_d = nc.dram_tensor("rv_slab", [65, JS], BF, kind="ExternalInput")
    bres_d = nc.dram_tensor(
        "bev_res_t", [128, (JS // 128) * C],
        mybir.dt.float32 if f32epi else BF, kind="ExternalInput")
    w_d = nc.dram_tensor("w_ext", [65, 2 * D + 65 + 64], BF, kind="ExternalInput")
    out_d = nc.dram_tensor(
        "out", [128, (JS // 128) * C],
        mybir.dt.float32 if f32epi else BF, kind="ExternalOutput")

    with tile.TileContext(nc) as tc:
        with (
            tc.tile_pool(name="const", bufs=1) as cpool,
            tc.tile_pool(name="ptile", bufs=6) as ppool,
            tc.tile_pool(name="epi", bufs=4) as xpool,
            tc.tile_pool(name="psum_e", bufs=3, space="PSUM") as epool,
            tc.tile_pool(name="psum_z", bufs=2, space="PSUM") as zpool,
        ):
          for _rep in range(reps):
            # ---- load inputs ----
            rv_sb = cpool.tile([65, N], BF, tag="rv")
            bev_sb = cpool.tile([65, N], BF, tag="bev")
            rvs_sb = cpool.tile([65, JS], BF, tag="rvs")
            brest_sb = cpool.tile(
                [128, (JS // 128) * C], F32 if f32epi else BF, tag="brest")
            w_sb = cpool.tile([65, 2 * D + 65 + 64], BF, tag="w")
            wq_sb = w_sb[:, 0:D]
            wk_sb = w_sb[:, D : 2 * D]
            wv_sb = w_sb[:, 2 * D : 2 * D + 65]
            # channels interleaved [0,4,1,5,2,6,3,7], replicated 4x so a
            # strip matmul writes its full 32-partition quadrant (the
            # replicas are never read; they exist so staged-mega copies only
            # read initialized PSUM). Cost is free-dim driven - 32-row lhsT
            # costs the same as 8.
            w32q_sb = w_sb[:, 81:113]
            w32k_sb = w_sb[:, 113:145]

            # DMA order = first-consumer order: w + bev blocks 0-3 on the
            # scalar HWDGE queue, the rest on sync, rv splits aligned to vt
            # mega boundaries.
            dmae = nc.scalar if os.environ.get("NO_SCALAR_DMA", "") != "1" else nc.sync
            dmae.dma_start(w_sb[:], w_d[:])
            dmae.dma_start(bev_sb[:, 0:2048], bev_d[:, 0:2048])
            nc.sync.dma_start(rvs_sb[:, 0:512], rvs_d[:, 0:512])
            nc.sync.dma_start(rv_sb[:, 0:1024], rv_d[:, 0:1024])
            nc.sync.dma_start(bev_sb[:, 2048:6144], bev_d[:, 2048:6144])
            nc.sync.dma_start(rv_sb[:, 1024:3072], rv_d[:, 1024:3072])
            nc.sync.dma_start(rvs_sb[:, 512:JS], rvs_d[:, 512:JS])
            nc.sync.dma_start(bev_sb[:, 6144:N], bev_d[:, 6144:N])
            nc.sync.dma_start(rv_sb[:, 3072:5120], rv_d[:, 3072:5120])
            nc.sync.dma_start(rv_sb[:, 5120:7168], rv_d[:, 5120:7168])
            nc.sync.dma_start(rv_sb[:, 7168:N], rv_d[:, 7168:N])
            nc.gpsimd.dma_start(brest_sb[:], bres_d[:])

            exp_bias = cpool.tile([128, 1], F32, tag="eb")
            nc.vector.memset(exp_bias[:], 1.0)
            if os.environ.get("NO_WARMUP", "") != "1":
                warm_ps = epool.tile([1, 16], F32, tag="e")
                for _w in range(3):
                    nc.tensor.matmul(
                        warm_ps[0:1, _w : _w + 1], exp_bias[0:1, :],
                        exp_bias[0:1, :], start=True, stop=True,
                    )

            # ---- attention operand tensors ----
            q8_sb = cpool.tile([4, 2, N], F8, tag="q8")
            k8_sb = cpool.tile([4, 2, JS], F8, tag="k8")
            vt_sb = cpool.tile([128, NT * 65], BF, tag="vt")
            stq_sb = cpool.tile([128, 2048], F8, tag="stq")
            stk_sb = cpool.tile([128, 512], F8, tag="stk")

            _ci = [0]

            def pcopy(out, in_, eng=None):
                if eng is None:
                    eng = "dve" if _ci[0] % 2 == 0 else "act"
                    _ci[0] += 1
                if eng == "dve":
                    nc.vector.tensor_copy(out, in_)
                else:
                    nc.scalar.copy(out, in_)

            # ---- projection producers ----
            def qk_old(w4, src, blk0, dst, eng=None):
                # direct path for the first blocks: [4, 1024] PSUM, one copy
                ps = epool.tile([4, 1024], F32, tag="e")
                nc.tensor.matmul(
                    ps[:, 0:512], w4[:, 0:4], src[:, blk0 : blk0 + 512],
                    start=True, stop=True,
                )
                nc.tensor.matmul(
                    ps[:, 512:1024], w4[:, 4:8], src[:, blk0 : blk0 + 512],
                    start=True, stop=True,
                )
                pcopy(dst[:, :, blk0 : blk0 + 512], ps[:], eng)

            # q strip megas: [128, 1024] PSUM holds 4 strips of 2 adjacent
            # 512-blocks ([8, 1024] at partition bases 0/32/64/96, 2 matmuls
            # each); one copy covers 8 blocks, one remap DMA per strip.
            QMEGA = [(4, 4), (12, 3)]   # (first block, n strips)

            def q_mega(m, eng=None):
                b0, nstrip = QMEGA[m]
                ps = epool.tile([128, 1024], F32, tag="e")
                for j in range(nstrip):
                    for h in range(2):
                        c0 = 512 * (b0 + 2 * j + h)
                        nc.tensor.matmul(
                            ps[32 * j : 32 * j + 32, 512 * h : 512 * (h + 1)],
                            w32q_sb[:], bev_sb[:, c0 : c0 + 512],
                            start=True, stop=True, tile_position=(0, 32 * j),
                        )
                if nstrip < 4:   # qB: fill the unused quadrant (never read)
                    for h in range(2):
                        nc.tensor.matmul(
                            ps[96:128, 512 * h : 512 * (h + 1)],
                            w32q_sb[:], bev_sb[:, 512 * h : 512 * (h + 1)],
                            start=True, stop=True, tile_position=(0, 96),
                        )
                pcopy(stq_sb[:, 1024 * m : 1024 * (m + 1)], ps[:], eng)

            def q_remaps(m):
                b0, nstrip = QMEGA[m]
                for j in range(nstrip):
                    c0 = 512 * (b0 + 2 * j)
                    nc.sync.dma_start(
                        q8_sb[:, :, c0 : c0 + 1024],
                        stq_sb[32 * j : 32 * j + 8, 1024 * m : 1024 * (m + 1)],
                    )

            def k_mega(eng=None):
                # staged k cols 512-2304 as 4 full-width quadrants; the last
                # one overlaps (cols 1792-2304) so all PSUM bytes are written.
                ps = epool.tile([128, 512], F32, tag="e")
                for j in range(4):
                    c0 = 512 * (1 + j) if j < 3 else JS - 512
                    nc.tensor.matmul(
                        ps[32 * j : 32 * j + 32, :], w32k_sb[:],
                        rvs_sb[:, c0 : c0 + 512],
                        start=True, stop=True, tile_position=(0, 32 * j),
                    )
                pcopy(stk_sb[:], ps[:], eng)

            def k_remaps():
                for j in range(3):
                    c0 = 512 * (1 + j)
                    nc.sync.dma_start(
                        k8_sb[:, :, c0 : c0 + 512],
                        stk_sb[32 * j : 32 * j + 8, :],
                    )
                nc.sync.dma_start(
                    k8_sb[:, :, 2048:JS],
                    stk_sb[96:104, 512 - (JS - 2048) : 512],
                )

            def vt_mega(m, eng=None):
                # 8 v^T chunks per PSUM tile / one copy; 4 chunks per
                # 512-f32 bank so no matmul crosses a bank boundary.
                ps = epool.tile([128, 2, 512], F32, tag="e")
                for cix in range(8):
                    t = 8 * m + cix
                    nc.tensor.matmul(
                        ps[:, cix // 4, (cix % 4) * 65 : (cix % 4 + 1) * 65],
                        rv_sb[:, t * 128 : (t + 1) * 128], wv_sb[:],
                        start=True, stop=True,
                    )
                pcopy(vt_sb[:, 520 * m : 520 * (m + 1)], ps[:, :, 0:260], eng)

            ALL_OLD = os.environ.get("ALL_OLD", "") == "1"

            def qk_old_pw(w4, src, blk0, pw, dst, eng=None):
                ps = epool.tile([4, 1024], F32, tag="e")
                nc.tensor.matmul(
                    ps[:, 0:pw], w4[:, 0:4], src[:, blk0 : blk0 + pw],
                    start=True, stop=True,
                )
                nc.tensor.matmul(
                    ps[:, 512 : 512 + pw], w4[:, 4:8], src[:, blk0 : blk0 + pw],
                    start=True, stop=True,
                )
                if pw == 512:
                    pcopy(dst[:, :, blk0 : blk0 + pw], ps[:], eng)
                else:
                    pcopy(dst[:, 0, blk0 : blk0 + pw], ps[:, 0:pw], eng)
                    pcopy(dst[:, 1, blk0 : blk0 + pw], ps[:, 512 : 512 + pw], eng)

            # ---- prologue producers ----
            # k_old's copy goes on ACT right before exp(0); q_old/vt0 copies
            # queue on DVE, which joins the exp rotation at step DVE_START.
            # k_old's copy is exp(0)'s gate - chain it on ACT.
            qk_old(wk_sb, rvs_sb, 0, k8_sb, "act")
            qk_old(wq_sb, bev_sb, 0, q8_sb, "dve")
            qk_old(wq_sb, bev_sb, 512, q8_sb, "dve")
            qk_old(wq_sb, bev_sb, 1024, q8_sb, "act")

            # step -> producer closures, emitted AFTER energy(i) so the
            # producer's PSUM slot follows E(i) in the rotation; the copy
            # engine matches exp(i+1) (and exp(i+3), same parity), so both
            # rotation waits are absorbed in that engine's in-order queue.
            if ALL_OLD:
                drip = {0: [lambda e: qk_old(wq_sb, bev_sb, 1536, q8_sb, e),
                            lambda e: vt_mega(0, e)]}
                for ii, blk in enumerate(range(4, 18)):
                    drip.setdefault(1 + 2 * ii, []).append(
                        (lambda b: lambda e: qk_old(wq_sb, bev_sb, 512 * b, q8_sb, e))(blk))
                for ii in range(1, 9):
                    drip.setdefault(2 + 3 * ii, []).append(
                        (lambda m: lambda e: vt_mega(m, e))(ii))
                drip.setdefault(4, []).append(
                    lambda e: qk_old_pw(wk_sb, rvs_sb, 512, 512, k8_sb, e))
                drip.setdefault(6, []).append(
                    lambda e: qk_old_pw(wk_sb, rvs_sb, 1024, 512, k8_sb, e))
                drip.setdefault(8, []).append(
                    lambda e: qk_old_pw(wk_sb, rvs_sb, 1536, 512, k8_sb, e))
                drip.setdefault(10, []).append(
                    lambda e: qk_old_pw(wk_sb, rvs_sb, 2048, 256, k8_sb, e))
            else:
              drip = {
                0: [lambda e: qk_old(wq_sb, bev_sb, 1536, q8_sb, e),
                    lambda e: vt_mega(0, e)],
                1: [lambda e: q_mega(0, e), lambda e: q_remaps(0)],
                3: [lambda e: vt_mega(1, e)],
                5: [lambda e: vt_mega(2, e)],
                7: [lambda e: k_mega(e), lambda e: k_remaps()],
                9: [lambda e: vt_mega(3, e)],
                11: [lambda e: q_mega(1, e), lambda e: q_remaps(1)],
                13: [lambda e: vt_mega(4, e)],
                17: [lambda e: vt_mega(5, e)],
                20: [lambda e: vt_mega(6, e)],
                23: [lambda e: vt_mega(7, e)],
                26: [lambda e: vt_mega(8, e)],
              }

            # ---- main attention loop ----
            groups = []
            for jb0, jbw in JBLOCKS[: int(os.environ.get("NJB", "5"))]:
                g = GW // jbw
                for grp in range(NT // g):
                    groups.append((jb0, jbw, g, grp))
            G = len(groups)

            zt_bufs = {}     # jb0 -> zt psum tile
            o_bufs = {}      # jb0 -> epilogue output tile
            e_tiles = {}
            p_tiles = {}
            epi_sched = {}   # step -> list of (jb0, jbw, s)

            def z_mms(p_tile, i):
                jb0, jbw, g, grp = groups[i]
                zt_buf = zt_bufs[jb0]
                nsub = jbw // 128
                if p_tile.dtype == I16:
                    p_tile = p_tile.bitcast(BF)
                for m in range(g):
                    t = grp * g + m
                    for s in range(nsub):
                        nc.tensor.matmul(
                            zt_buf[:, s * 65 : s * 65 + 65],
                            p_tile[:, m, s * 128 : (s + 1) * 128],
                            vt_sb[:, t * 65 : (t + 1) * 65],
                            start=(t == 0 and s == 0),
                            stop=(t == NT - 1 and s == nsub - 1),
                        )

            nsteps = G + 3 + 3 + max(jbw // 128 for _, jbw in JBLOCKS)
            for i in range(nsteps):
                if i < G:
                    jb0, jbw, g, grp = groups[i]
                    if grp == 0:
                        zt_new = zpool.tile([128, 512], F32, tag="zt")
                        zt_bufs[jb0] = zt_new
                    e_ps = epool.tile([128, GW], F32, tag="e")
                    e_tiles[i] = e_ps
                    for m in range(g):
                        t = grp * g + m
                        for pc0 in range(0, jbw, 256):
                            pw = min(256, jbw - pc0)
                            nc.tensor.matmul(
                                e_ps[:, m * jbw + pc0 : m * jbw + pc0 + pw],
                                q8_sb[:, :, t * 128 : (t + 1) * 128],
                                k8_sb[:, :, jb0 + pc0 : jb0 + pc0 + pw],
                                start=True, stop=True,
                                perf_mode=DR,
                            )
                peng = "dve" if exp_on_dve(i + 1) else "act"
                for fn in drip.pop(i, ()):
                    fn(peng)
                if 0 <= i - 1 < G:
                    jb0, jbw, g, grp = groups[i - 1]
                    e_prev = e_tiles.pop(i - 1)
                    pshape = [128, g, jbw]
                    if exp_on_dve(i - 1):
                        # DVE fast-exp: int16 bit pattern = bf16 exp(x)
                        p_sb = ppool.tile(pshape, I16, tag="p")
                        nc.vector.tensor_scalar(
                            p_sb[:], e_prev[:], EXP_A, EXP_B, Mul, Add,
                        )
                    else:
                        p_sb = ppool.tile(pshape, BF, tag="p")
                        nc.scalar.activation(p_sb[:], e_prev[:], Exp)
                    p_tiles[i - 1] = p_sb
                if 0 <= i - 3 < G:
                    z_mms(p_tiles.pop(i - 3), i - 3)
                    jb0, jbw, g, grp = groups[i - 3]
                    if grp == NT // g - 1:  # jblock's z complete
                        lag = 0 if i - 3 >= G - 1 else 2
                        for s in range(jbw // 128):
                            epi_sched.setdefault(i + lag + s, []).append(
                                (jb0, jbw, s)
                            )
                # ---- normalize + residual + store (per j-128-subblock) ----
                for jb0, jbw, s in epi_sched.pop(i, ()):
                    nsub = jbw // 128
                    zt = zt_bufs[jb0][:, s * 65 : s * 65 + 65]
                    jb = jb0 // 128 + s
                    if s == 0:
                        o_new = xpool.tile(
                            [128, 4 * C], F32 if f32epi else BF, tag="o")
                        o_bufs[jb0] = o_new
                    o_sb = o_bufs[jb0]
                    r_sb = xpool.tile([128, 1], F32, tag="r")
                    nc.vector.reciprocal(r_sb[:], zt[:, 64:65])
                    nc.scalar.activation(
                        o_sb[:, s * C : (s + 1) * C], zt[:, 0:64],
                        Copy, scale=r_sb[:],
                    )
                    nc.gpsimd.tensor_add(
                        o_sb[:, s * C : (s + 1) * C],
                        o_sb[:, s * C : (s + 1) * C],
                        brest_sb[:, jb * C : (jb + 1) * C],
                    )
                    if s == nsub - 1:  # one store per jblock
                        nc.sync.dma_start(
                            out_d[:, (jb0 // 128) * C : (jb + 1) * C],
                            o_bufs.pop(jb0)[:, : nsub * C],
                        )

    nc.compile()
    return nc


def get_program(reps=1, dve_share=None):
    key = (reps, dve_share)
    if key not in _PROGRAMS:
        _PROGRAMS[key] = _build_program(reps, dve_share)
    return _PROGRAMS[key]


def make_in_maps(rv_x, bev_x, Wq, bq, Wk, bk, Wv, bv):
    rv_x = np.asarray(rv_x, np.float32)
    bev_x = np.asarray(bev_x, np.float32)
    ones = np.ones((1, N), np.float32)
    wq_ext = np.concatenate([np.asarray(Wq).T, np.asarray(bq)[None]], 0)
    wk_ext = np.concatenate([np.asarray(Wk).T, np.asarray(bk)[None]], 0)
    wv_ext = np.zeros((65, 65), np.float32)
    wv_ext[:64, :64] = np.asarray(Wv).T
    wv_ext[64, :64] = np.asarray(bv)
    wv_ext[64, 64] = 1.0
    perm = [0, 4, 1, 5, 2, 6, 3, 7] * 4  # partition p = 2r+s, replicated 4x
    w_ext = np.concatenate(
        [wq_ext, wk_ext, wv_ext, wq_ext[:, perm], wk_ext[:, perm]], 1
    ).astype(BF16)

    in_maps = []
    for core in range(NCORES):
        b = core // 4
        j0 = (core % 4) * JS
        rv2 = rv_x[b].reshape(C, N)
        bev2 = bev_x[b].reshape(C, N)
        rv_ext = np.concatenate([rv2, ones], 0).astype(BF16)
        bev_ext = np.concatenate([bev2, ones], 0).astype(BF16)
        # residual pre-swizzled to the kernel's SBUF layout [p, (jblk, c)]
        brest = (
            bev2[:, j0 : j0 + JS].T.reshape(JS // 128, 128, C)
            .transpose(1, 0, 2).reshape(128, -1)
        ).astype(np.float32 if os.environ.get("F32_EPI", "") == "1" else BF16)
        in_maps.append(
            {
                "rv_ext": rv_ext,
                "bev_ext": bev_ext,
                "rv_slab": np.ascontiguousarray(rv_ext[:, j0 : j0 + JS]),
                "bev_res_t": np.ascontiguousarray(brest),
                "w_ext": w_ext,
            }
        )
    return in_maps


def unswizzle_out(arr):
    """[128, (jblk, c)] device layout -> [C, JS] slab (f32)."""
    return (
        np.asarray(arr).astype(np.float32)
        .reshape(128, JS // 128, C).transpose(1, 0, 2).reshape(JS, C).T
    )


def run(inputs, trace=False, trace_kwargs=None, reps=1, in_maps=None):
    """Run on all 8 cores; returns (output ndarray, BassKernelResults)."""
    from concourse.bass_utils import run_bass_kernel_spmd

    nc = get_program(reps)
    if in_maps is None:
        in_maps = make_in_maps(**inputs)
    res = run_bass_kernel_spmd(
        nc,
        in_maps,
        core_ids=list(range(NCORES)),
        trace=trace,
        **(trace_kwargs or {}),
    )
    out = np.zeros((B, C, N), np.float32)
    for core in range(NCORES):
        b = core // 4
        j0 = (core % 4) * JS
        out[b, :, j0 : j0 + JS] = unswizzle_out(res.results[core]["out"])
    return out.reshape(B, C, H, W), res


def kernel(**inputs):
    out, _ = run(inputs)
    return out
